# revision 13
# baseline (speedup 1.0000x reference)
"""DiverseBeamSearch kernel for 8 trn2 NeuronCores.

Strategy: the heavy op is a top-k over lprobs [64, 8, 50257] f32 (~103 MB,
memory-bound). Each (batch, beam) row is consumed by exactly one beam group,
the diversity penalty only decreases <=6 known columns per row, masked rows
become constants, and the accumulated-score add is uniform per row — so the
exact result is recoverable from each row's exact top-8 raw columns.

Device (per core, 64 rows = 8 batches x 8 beams): stream the padded rows as
[128 partitions x 25216] f32, per column-tile run an exact f32 pairwise-max
fold tree 3152 -> 197, then max8 + max_index per fold segment. This yields,
for every 1/16th row segment, the top-8 fold groups (value + position) — a
guaranteed superset of the row's true top-8 columns. Host expands candidate
groups, applies penalty/mask/score to candidates only, and reproduces the
reference's top-k, tie-breaking and overlap update bit-exactly.
"""
import numpy as np

import concourse.bass as bass
import concourse.mybir as mybir
from concourse.tile import TileContext
from concourse.vector_clock import ScopedClock
from concourse.bass_utils import run_bass_kernel_spmd

# ---- problem constants (hardcoded per contract) ----
BSZ, BEAM, VOCAB = 64, 8, 50257
NUM_GROUPS = 4
MB = BEAM // NUM_GROUPS          # 2
K = MB                           # CAND_MULT = 1
N_GRAM = 2
DIVERSITY_STRENGTH = np.float32(-0.5)
DIVERSITY_DISCOUNT = np.float32(0.5)
NCORES = 8

# ---- device layout ----
VPAD = 50432                     # padded row, = 2 * HALF
HALF = 25216                     # per-partition span (2 partitions per row)
# Tapered column tiles: small first (compute starts early), small last
# (short post-stream fold tail). Must sum to HALF; each divisible by FOLDW.
TILES = (1576, 3152, 6304, 6304, 6304, 1576)
NTILE = len(TILES)
BASES = tuple(int(x) for x in np.cumsum((0,) + TILES[:-1]))
FOLDW = 394                      # fold-tree leaf width
MAXMEM = max(TILES) // FOLDW     # max members per fold group (16)
LD_BUFS = 6
ROWS_PER_CORE = BSZ // NCORES * BEAM   # 64
NEG = np.float32(-3.0e38)

_OPMAP = {"sem-ge-imm": "sem-ge", "sem-eq-imm": "sem-eq"}


class _TileContextSplitDrain(TileContext):
    """This walrus build rejects a Drain carrying >1 sync-wait command; emit
    the tail-drain waits as individual nop waits on SP instead."""

    def _drain_and_barrier(self, tick_clock, wait_clock):
        nc = self.nc
        probe = nc.sync.nop(nofuse=True)
        wait_clock.add_sem_waits(
            probe.ins, ScopedClock({None: tick_clock.global_clock})
        )
        waits = list(probe.ins.sync_info.on_wait) if probe.ins.sync_info else []
        probe.ins.sync_info = None
        name2sem = {h.name: h for h in self.sems.allocated().values()}
        for w in waits:
            nc.sync.nop(nofuse=True).wait_op(
                name2sem[w.ant_name], w.wait_value, _OPMAP[w.wait_mode]
            )
        nc.sync.drain()
        nc.all_engine_barrier()
        popped = nc._tile_sem_poison_stack.pop()
        assert popped is self._sem_poison
        nc.clear_and_free_semaphores(list(self.sems.allocated().values()))
        nc.all_engine_barrier()


def _split_multiwait_instructions(nc):
    """walrus here rejects any instruction carrying >1 sync-wait command;
    hoist all but one wait onto same-engine nop instructions just before."""
    seq = 0
    for f in nc.m.functions:
        for bb in f.blocks:
            out = []
            for inst in bb.instructions:
                waits = list(inst.sync_info.on_wait) if inst.sync_info else []
                if len(waits) > 1:
                    for w in waits[:-1]:
                        nop = mybir.InstNoOp(name=f"splitwait-{seq}", ins=[],
                                             outs=[])
                        seq += 1
                        nop.engine = inst.engine
                        nop.sync_info = mybir.SyncInfo(on_wait=[w],
                                                       on_update=[])
                        out.append(nop)
                    inst.sync_info.on_wait = [waits[-1]]
                out.append(inst)
            bb.instructions = out


def build_nc():
    nc = bass.Bass()
    lp = nc.dram_tensor("lp", [128, HALF], mybir.dt.float32, kind="ExternalInput")
    vals = nc.dram_tensor("vals", [128, NTILE * 8], mybir.dt.bfloat16,
                          kind="ExternalOutput")
    idx = nc.dram_tensor("idx", [128, NTILE * 8], mybir.dt.uint32,
                         kind="ExternalOutput")
    with _TileContextSplitDrain(nc) as tc:
        with tc.tile_pool(name="io", bufs=1) as iop, \
             tc.tile_pool(name="ld", bufs=LD_BUFS) as ldp, \
             tc.tile_pool(name="fw", bufs=2) as fwp:
            vout = iop.tile([128, NTILE * 8], mybir.dt.bfloat16)
            iout = iop.tile([128, NTILE * 8], mybir.dt.uint32)
            for t, tilew in enumerate(TILES):
                base = BASES[t]
                tl = ldp.tile([128, tilew], mybir.dt.bfloat16)
                # SWDGE cast-DMA: HBM f32 -> SBUF bf16 (monotone rounding);
                # halves SBUF footprint and enables 2x DVE fold mode.
                nc.gpsimd.dma_start(out=tl[:, :], in_=lp[:, base:base + tilew])
                w = tilew
                src = tl
                while w > FOLDW:
                    w //= 2
                    dst = fwp.tile([128, w], mybir.dt.bfloat16, tag=f"fold{w}")
                    nc.vector.tensor_tensor(out=dst[:, :], in0=src[:, :w],
                                            in1=src[:, w:2 * w],
                                            op=mybir.AluOpType.max)
                    src = dst
                nc.vector.max(out=vout[:, t * 8:(t + 1) * 8], in_=src[:, :])
                nc.vector.max_index(out=iout[:, t * 8:(t + 1) * 8],
                                    in_max=vout[:, t * 8:(t + 1) * 8],
                                    in_values=src[:, :])
            nc.sync.dma_start(out=vals[:, :], in_=vout[:, :])
            nc.sync.dma_start(out=idx[:, :], in_=iout[:, :])
    _split_multiwait_instructions(nc)
    return nc


_NC_CACHE = None


def _get_nc():
    global _NC_CACHE
    if _NC_CACHE is None:
        _NC_CACHE = build_nc()
    return _NC_CACHE


def _run_device(lprobs, trace=False, tmpdir=None):
    """lprobs [64, 8, 50257] f32 -> vals [512, 2, NTILE, 8] f32,
    cols [512, 2, NTILE, 8] i64, plus the raw results object."""
    rows = np.ascontiguousarray(lprobs, dtype=np.float32).reshape(
        BSZ * BEAM, VOCAB)
    pad = np.full((BSZ * BEAM, VPAD - VOCAB), NEG, np.float32)
    x = np.concatenate([rows, pad], axis=1).reshape(NCORES, 128, HALF)
    in_maps = [{"lp": x[c]} for c in range(NCORES)]
    res = run_bass_kernel_spmd(_get_nc(), in_maps,
                               core_ids=list(range(NCORES)), trace=trace,
                               tmpdir=tmpdir)
    vals = np.stack([np.asarray(res.results[c]["vals"], dtype=np.float32)
                     for c in range(NCORES)])
    cols = np.stack([res.results[c]["idx"] for c in range(NCORES)])
    vals = vals.reshape(BSZ * BEAM, 2, NTILE, 8)
    cols = cols.astype(np.int64).reshape(BSZ * BEAM, 2, NTILE, 8)
    return vals, cols, res


def _expand_candidates(vals, cols, topg=16):
    """-> candidate vocab col ids [R, topg*MAXMEM] (int64, -1 for padding)."""
    R = vals.shape[0]
    v = vals.reshape(R, 2 * NTILE * 8)
    h = np.arange(2)[:, None, None]
    t_base = np.asarray(BASES)[None, :, None]
    t_end = (np.asarray(BASES) + np.asarray(TILES))[None, :, None]
    base = (h * HALF + t_base).astype(np.int64)
    end = (h * HALF + t_end).astype(np.int64)
    c0 = (cols + base).reshape(R, 2 * NTILE * 8)
    ce = np.broadcast_to(end, (R, 2, NTILE, 8)).reshape(R, 2 * NTILE * 8)
    top = np.argpartition(-v, topg, axis=1)[:, :topg]
    cbase = np.take_along_axis(c0, top, axis=1)
    cend = np.take_along_axis(ce, top, axis=1)
    members = cbase[:, :, None] + np.arange(MAXMEM)[None, None, :] * FOLDW
    ok = (members < cend[:, :, None]) & (members < VOCAB)
    members = np.where(ok, members, -1)
    return members.reshape(R, topg * MAXMEM)


def _host_merge(lprobs, scores, group_overlap, original_batch_idxs,
                mask_last_n_gram_indices, step, vals, cols):
    bsz, beam, vocab = BSZ, BEAM, VOCAB
    G, mb, k = NUM_GROUPS, MB, K
    flat_rows = np.asarray(lprobs, dtype=np.float32).reshape(bsz * beam, vocab)
    cand = _expand_candidates(vals, cols)
    R = cand.shape[0]
    safe = np.where(cand >= 0, cand, 0)
    cvals = flat_rows[np.arange(R)[:, None], safe]
    cvals = np.where(cand >= 0, cvals, NEG).astype(np.float32)

    obi = np.asarray(original_batch_idxs)
    go = np.asarray(group_overlap, dtype=np.float32)
    mask = np.asarray(mask_last_n_gram_indices)
    sc_all = np.asarray(scores, dtype=np.float32)[:, :, step - 1]

    scores_G = np.zeros((bsz, G, k), np.float32)
    idx_G = np.zeros((bsz, G, k), np.int32)
    beams_G = np.zeros((bsz, G, k), np.int32)

    for b in range(bsz):
        for g in range(G):
            div = {}
            if g > 0:
                pen = np.float32(1.0) + go[obi[b], g, :g]
                for m in range(mb):
                    for j in range(g):
                        tok = int(idx_G[b, j, m])
                        div[tok] = np.float32(
                            div.get(tok, np.float32(0.0)) + pen[j])
            ents = []
            for m in range(mb):
                bm = g + m * G
                row = b * beam + bm
                sc = np.float32(sc_all[b, bm])
                if mask[b, bm, :].sum() != N_GRAM:
                    ents.append((np.float32(sc), m * vocab + 0))
                    ents.append((np.float32(sc), m * vocab + 1))
                    continue
                cc = cand[row]
                cv = cvals[row]
                seen = set()
                for ci in range(cc.shape[0]):
                    c = int(cc[ci])
                    if c < 0 or c in seen:
                        continue
                    seen.add(c)
                    v = cv[ci]
                    if c in div:
                        v = np.float32(v + np.float32(DIVERSITY_STRENGTH * div[c]))
                    ents.append((np.float32(v + sc), m * vocab + c))
            ents.sort(key=lambda e: (-e[0], e[1]))
            for r_ in range(k):
                v, fp = ents[r_]
                scores_G[b, g, r_] = v
                idx_G[b, g, r_] = fp % vocab
                beams_G[b, g, r_] = (fp // vocab) * G + g

    scores_buf = np.transpose(scores_G, (0, 2, 1)).reshape(bsz, beam)
    indices_buf = np.transpose(idx_G, (0, 2, 1)).reshape(bsz, beam)
    beams_buf = np.transpose(beams_G, (0, 2, 1)).reshape(bsz, beam)
    return scores_buf, indices_buf, beams_buf


def _overlap_update(group_overlap, original_batch_idxs, last_n_gram_indices,
                    mask_last_n_gram_indices, indices_buf):
    bsz, beam = BSZ, BEAM
    G, mb = NUM_GROUPS, MB
    go = np.asarray(group_overlap, dtype=np.float32)
    obi = np.asarray(original_batch_idxs)
    lng = np.asarray(last_n_gram_indices)
    mask = np.asarray(mask_last_n_gram_indices)
    present = np.concatenate([lng, indices_buf[:, :, None]], axis=-1)
    present = present.reshape(bsz, mb, G, N_GRAM)
    eq = present[:, :, None, :, :] == present[:, :, :, None, :]
    m_ = (mask != 0).reshape(bsz, mb, G, N_GRAM)
    om = m_[:, :, None, :, :] & m_[:, :, :, None, :]
    eq = np.where(om, eq, False)
    ov = (eq.sum(-1) == N_GRAM).astype(np.int32).sum(1)
    ngo = go.copy()
    np.add.at(ngo, obi, ov.astype(ngo.dtype))
    return ngo * DIVERSITY_DISCOUNT


def kernel(lprobs, scores, group_overlap, original_batch_idxs,
           last_n_gram_indices, mask_last_n_gram_indices, step,
           _trace=False, _ret_res=False, _tmpdir=None):
    step = int(step)
    vals, cols, res = _run_device(lprobs, trace=_trace, tmpdir=_tmpdir)
    scores_buf, indices_buf, beams_buf = _host_merge(
        lprobs, scores, group_overlap, original_batch_idxs,
        mask_last_n_gram_indices, step, vals, cols)
    new_go = _overlap_update(group_overlap, original_batch_idxs,
                             last_n_gram_indices, mask_last_n_gram_indices,
                             indices_buf)
    out = (scores_buf, indices_buf.astype(np.int32),
           beams_buf.astype(np.int32), new_go)
    if _ret_res:
        return out, res
    return out


# revision 14
# speedup vs baseline: 1.1359x; 1.1359x over previous
"""DiverseBeamSearch kernel for 8 trn2 NeuronCores.

Strategy: the heavy op is a top-k over lprobs [64, 8, 50257] f32 (~103 MB,
memory-bound). Each (batch, beam) row is consumed by exactly one beam group,
the diversity penalty only decreases <=6 known columns per row, masked rows
become constants, and the accumulated-score add is uniform per row — so the
exact result is recoverable from each row's exact top-8 raw columns.

Device (per core, 64 rows = 8 batches x 8 beams): stream the padded rows as
[128 partitions x 25216] f32, per column-tile run an exact f32 pairwise-max
fold tree 3152 -> 197, then max8 + max_index per fold segment. This yields,
for every 1/16th row segment, the top-8 fold groups (value + position) — a
guaranteed superset of the row's true top-8 columns. Host expands candidate
groups, applies penalty/mask/score to candidates only, and reproduces the
reference's top-k, tie-breaking and overlap update bit-exactly.
"""
import numpy as np

import concourse.bass as bass
import concourse.mybir as mybir
from concourse.tile import TileContext
from concourse.vector_clock import ScopedClock
from concourse.bass_utils import run_bass_kernel_spmd

# ---- problem constants (hardcoded per contract) ----
BSZ, BEAM, VOCAB = 64, 8, 50257
NUM_GROUPS = 4
MB = BEAM // NUM_GROUPS          # 2
K = MB                           # CAND_MULT = 1
N_GRAM = 2
DIVERSITY_STRENGTH = np.float32(-0.5)
DIVERSITY_DISCOUNT = np.float32(0.5)
NCORES = 8

# ---- device layout ----
VPAD = 50432                     # padded row, = 2 * HALF
HALF = 25216                     # per-partition span (2 partitions per row)
# Tapered column tiles: small first (compute starts early), small last
# (short post-stream fold tail). Must sum to HALF; each divisible by FOLDW.
TILES = (1576, 6304, 6304, 6304, 3152, 1576)
NTILE = len(TILES)
BASES = tuple(int(x) for x in np.cumsum((0,) + TILES[:-1]))
FOLDW = 394                      # fold-tree leaf width
MAXMEM = max(TILES) // FOLDW     # max members per fold group (16)
LD_BUFS = 6
ROWS_PER_CORE = BSZ // NCORES * BEAM   # 64
NEG = np.float32(-3.0e38)

_OPMAP = {"sem-ge-imm": "sem-ge", "sem-eq-imm": "sem-eq"}


class _TileContextSplitDrain(TileContext):
    """This walrus build rejects a Drain carrying >1 sync-wait command; emit
    the tail-drain waits as individual nop waits on SP instead."""

    def _drain_and_barrier(self, tick_clock, wait_clock):
        nc = self.nc
        probe = nc.sync.nop(nofuse=True)
        wait_clock.add_sem_waits(
            probe.ins, ScopedClock({None: tick_clock.global_clock})
        )
        waits = list(probe.ins.sync_info.on_wait) if probe.ins.sync_info else []
        probe.ins.sync_info = None
        name2sem = {h.name: h for h in self.sems.allocated().values()}
        for w in waits:
            nc.sync.nop(nofuse=True).wait_op(
                name2sem[w.ant_name], w.wait_value, _OPMAP[w.wait_mode]
            )
        nc.sync.drain()
        nc.all_engine_barrier()
        popped = nc._tile_sem_poison_stack.pop()
        assert popped is self._sem_poison
        nc.clear_and_free_semaphores(list(self.sems.allocated().values()))
        nc.all_engine_barrier()


def _split_multiwait_instructions(nc):
    """walrus here rejects any instruction carrying >1 sync-wait command;
    hoist all but one wait onto same-engine nop instructions just before."""
    seq = 0
    for f in nc.m.functions:
        for bb in f.blocks:
            out = []
            for inst in bb.instructions:
                waits = list(inst.sync_info.on_wait) if inst.sync_info else []
                if len(waits) > 1:
                    for w in waits[:-1]:
                        nop = mybir.InstNoOp(name=f"splitwait-{seq}", ins=[],
                                             outs=[])
                        seq += 1
                        nop.engine = inst.engine
                        nop.sync_info = mybir.SyncInfo(on_wait=[w],
                                                       on_update=[])
                        out.append(nop)
                    inst.sync_info.on_wait = [waits[-1]]
                out.append(inst)
            bb.instructions = out


def build_nc():
    nc = bass.Bass()
    lp = nc.dram_tensor("lp", [128, HALF], mybir.dt.float32, kind="ExternalInput")
    vals = nc.dram_tensor("vals", [128, NTILE * 8], mybir.dt.bfloat16,
                          kind="ExternalOutput")
    idx = nc.dram_tensor("idx", [128, NTILE * 8], mybir.dt.uint32,
                         kind="ExternalOutput")
    with _TileContextSplitDrain(nc) as tc:
        with tc.tile_pool(name="io", bufs=1) as iop, \
             tc.tile_pool(name="ld", bufs=LD_BUFS) as ldp, \
             tc.tile_pool(name="fw", bufs=2) as fwp:
            vout = iop.tile([128, NTILE * 8], mybir.dt.bfloat16)
            iout = iop.tile([128, NTILE * 8], mybir.dt.uint32)
            for t, tilew in enumerate(TILES):
                base = BASES[t]
                tl = ldp.tile([128, tilew], mybir.dt.bfloat16)
                # SWDGE cast-DMA: HBM f32 -> SBUF bf16 (monotone rounding);
                # halves SBUF footprint and enables 2x DVE fold mode.
                nc.gpsimd.dma_start(out=tl[:, :], in_=lp[:, base:base + tilew])
                w = tilew
                src = tl
                while w > FOLDW:
                    w //= 2
                    dst = fwp.tile([128, w], mybir.dt.bfloat16, tag=f"fold{w}")
                    nc.vector.tensor_tensor(out=dst[:, :], in0=src[:, :w],
                                            in1=src[:, w:2 * w],
                                            op=mybir.AluOpType.max)
                    src = dst
                nc.vector.max(out=vout[:, t * 8:(t + 1) * 8], in_=src[:, :])
                nc.vector.max_index(out=iout[:, t * 8:(t + 1) * 8],
                                    in_max=vout[:, t * 8:(t + 1) * 8],
                                    in_values=src[:, :])
                nc.scalar.dma_start(out=vals[:, t * 8:(t + 1) * 8],
                                    in_=vout[:, t * 8:(t + 1) * 8])
                nc.scalar.dma_start(out=idx[:, t * 8:(t + 1) * 8],
                                    in_=iout[:, t * 8:(t + 1) * 8])
    _split_multiwait_instructions(nc)
    return nc


_NC_CACHE = None


def _get_nc():
    global _NC_CACHE
    if _NC_CACHE is None:
        _NC_CACHE = build_nc()
    return _NC_CACHE


def _run_device(lprobs, trace=False, tmpdir=None):
    """lprobs [64, 8, 50257] f32 -> vals [512, 2, NTILE, 8] f32,
    cols [512, 2, NTILE, 8] i64, plus the raw results object."""
    rows = np.ascontiguousarray(lprobs, dtype=np.float32).reshape(
        BSZ * BEAM, VOCAB)
    pad = np.full((BSZ * BEAM, VPAD - VOCAB), NEG, np.float32)
    x = np.concatenate([rows, pad], axis=1).reshape(NCORES, 128, HALF)
    in_maps = [{"lp": x[c]} for c in range(NCORES)]
    res = run_bass_kernel_spmd(_get_nc(), in_maps,
                               core_ids=list(range(NCORES)), trace=trace,
                               tmpdir=tmpdir)
    vals = np.stack([np.asarray(res.results[c]["vals"], dtype=np.float32)
                     for c in range(NCORES)])
    cols = np.stack([res.results[c]["idx"] for c in range(NCORES)])
    vals = vals.reshape(BSZ * BEAM, 2, NTILE, 8)
    cols = cols.astype(np.int64).reshape(BSZ * BEAM, 2, NTILE, 8)
    return vals, cols, res


def _expand_candidates(vals, cols, topg=16):
    """-> candidate vocab col ids [R, topg*MAXMEM] (int64, -1 for padding)."""
    R = vals.shape[0]
    v = vals.reshape(R, 2 * NTILE * 8)
    h = np.arange(2)[:, None, None]
    t_base = np.asarray(BASES)[None, :, None]
    t_end = (np.asarray(BASES) + np.asarray(TILES))[None, :, None]
    base = (h * HALF + t_base).astype(np.int64)
    end = (h * HALF + t_end).astype(np.int64)
    c0 = (cols + base).reshape(R, 2 * NTILE * 8)
    ce = np.broadcast_to(end, (R, 2, NTILE, 8)).reshape(R, 2 * NTILE * 8)
    top = np.argpartition(-v, topg, axis=1)[:, :topg]
    cbase = np.take_along_axis(c0, top, axis=1)
    cend = np.take_along_axis(ce, top, axis=1)
    members = cbase[:, :, None] + np.arange(MAXMEM)[None, None, :] * FOLDW
    ok = (members < cend[:, :, None]) & (members < VOCAB)
    members = np.where(ok, members, -1)
    return members.reshape(R, topg * MAXMEM)


def _host_merge(lprobs, scores, group_overlap, original_batch_idxs,
                mask_last_n_gram_indices, step, vals, cols):
    bsz, beam, vocab = BSZ, BEAM, VOCAB
    G, mb, k = NUM_GROUPS, MB, K
    flat_rows = np.asarray(lprobs, dtype=np.float32).reshape(bsz * beam, vocab)
    cand = _expand_candidates(vals, cols)
    R = cand.shape[0]
    safe = np.where(cand >= 0, cand, 0)
    cvals = flat_rows[np.arange(R)[:, None], safe]
    cvals = np.where(cand >= 0, cvals, NEG).astype(np.float32)

    obi = np.asarray(original_batch_idxs)
    go = np.asarray(group_overlap, dtype=np.float32)
    mask = np.asarray(mask_last_n_gram_indices)
    sc_all = np.asarray(scores, dtype=np.float32)[:, :, step - 1]

    scores_G = np.zeros((bsz, G, k), np.float32)
    idx_G = np.zeros((bsz, G, k), np.int32)
    beams_G = np.zeros((bsz, G, k), np.int32)

    for b in range(bsz):
        for g in range(G):
            div = {}
            if g > 0:
                pen = np.float32(1.0) + go[obi[b], g, :g]
                for m in range(mb):
                    for j in range(g):
                        tok = int(idx_G[b, j, m])
                        div[tok] = np.float32(
                            div.get(tok, np.float32(0.0)) + pen[j])
            ents = []
            for m in range(mb):
                bm = g + m * G
                row = b * beam + bm
                sc = np.float32(sc_all[b, bm])
                if mask[b, bm, :].sum() != N_GRAM:
                    ents.append((np.float32(sc), m * vocab + 0))
                    ents.append((np.float32(sc), m * vocab + 1))
                    continue
                cc = cand[row]
                cv = cvals[row]
                seen = set()
                for ci in range(cc.shape[0]):
                    c = int(cc[ci])
                    if c < 0 or c in seen:
                        continue
                    seen.add(c)
                    v = cv[ci]
                    if c in div:
                        v = np.float32(v + np.float32(DIVERSITY_STRENGTH * div[c]))
                    ents.append((np.float32(v + sc), m * vocab + c))
            ents.sort(key=lambda e: (-e[0], e[1]))
            for r_ in range(k):
                v, fp = ents[r_]
                scores_G[b, g, r_] = v
                idx_G[b, g, r_] = fp % vocab
                beams_G[b, g, r_] = (fp // vocab) * G + g

    scores_buf = np.transpose(scores_G, (0, 2, 1)).reshape(bsz, beam)
    indices_buf = np.transpose(idx_G, (0, 2, 1)).reshape(bsz, beam)
    beams_buf = np.transpose(beams_G, (0, 2, 1)).reshape(bsz, beam)
    return scores_buf, indices_buf, beams_buf


def _overlap_update(group_overlap, original_batch_idxs, last_n_gram_indices,
                    mask_last_n_gram_indices, indices_buf):
    bsz, beam = BSZ, BEAM
    G, mb = NUM_GROUPS, MB
    go = np.asarray(group_overlap, dtype=np.float32)
    obi = np.asarray(original_batch_idxs)
    lng = np.asarray(last_n_gram_indices)
    mask = np.asarray(mask_last_n_gram_indices)
    present = np.concatenate([lng, indices_buf[:, :, None]], axis=-1)
    present = present.reshape(bsz, mb, G, N_GRAM)
    eq = present[:, :, None, :, :] == present[:, :, :, None, :]
    m_ = (mask != 0).reshape(bsz, mb, G, N_GRAM)
    om = m_[:, :, None, :, :] & m_[:, :, :, None, :]
    eq = np.where(om, eq, False)
    ov = (eq.sum(-1) == N_GRAM).astype(np.int32).sum(1)
    ngo = go.copy()
    np.add.at(ngo, obi, ov.astype(ngo.dtype))
    return ngo * DIVERSITY_DISCOUNT


def kernel(lprobs, scores, group_overlap, original_batch_idxs,
           last_n_gram_indices, mask_last_n_gram_indices, step,
           _trace=False, _ret_res=False, _tmpdir=None):
    step = int(step)
    vals, cols, res = _run_device(lprobs, trace=_trace, tmpdir=_tmpdir)
    scores_buf, indices_buf, beams_buf = _host_merge(
        lprobs, scores, group_overlap, original_batch_idxs,
        mask_last_n_gram_indices, step, vals, cols)
    new_go = _overlap_update(group_overlap, original_batch_idxs,
                             last_n_gram_indices, mask_last_n_gram_indices,
                             indices_buf)
    out = (scores_buf, indices_buf.astype(np.int32),
           beams_buf.astype(np.int32), new_go)
    if _ret_res:
        return out, res
    return out
